# revision 17
# baseline (speedup 1.0000x reference)
"""AdaptiveWingLoss on 8 TRN2 NeuronCores.

Math (theta=0.5, eps=1, alpha=2.1, omega=14):
  d  = |x - y|,  p = 2.1 - y,  pm = y - 2.1,  z = ln2*pm
  nl  = log1p(d^p) = Ln(exp(p*ln d) + 1)
  lp  = log1p(e^z),  sigma = 1/(1+e^-z),  ps = p*sigma
  lin = ps*(2d-1) + lp = A'(y)*(d-0.5) + lp,  A' = 2*ps = -2*pm*sigma(z)
  loss/14 = select(d<0.5, nl, lin)

Key identity: d >= 0.5  <=>  nl >= lp (monotone), so the select dissolves:
  sum(loss)/14 = sum(nl) + sum(A' * relu(d-0.5)) + sum(min(lp - nl, 0))

sigma(z) and lp are approximated by quadratics in pm (max rel err ~1e-3,
end-to-end ~3e-6), evaluated inside fused custom DVE ops with accum=ADD.
All ACT work is in the single {ln, exp} table set (no table switches).
Batch dim sharded across 8 cores; each core emits per-partition partial
sums [128,1]; host adds and scales by 14.
"""
import numpy as np

import concourse.bacc as bacc
import concourse.mybir as mybir
import concourse.dve_ops as dops
from concourse.dve_spec import Spec, Src0, Src1, C0, C1, C2, Zero, lower, maxx, minn, relu, _has_src1
from concourse.dve_uop import DveOpSpec
from concourse.tile import TileContext
from concourse.bass_utils import run_bass_kernel_spmd

N_CORES = 8
ROWS, COLS = 1024, 2048  # per-core shard, fp32 elements
NT = ROWS // 128
LN2 = float(np.log(2.0))
D_EPS = 1e-6

# quadratic fits in y on [0, 1] (np.polyfit deg 2; end-to-end err ~5e-5)
# A'(y) = -2*(y-2.1)*sigma(ln2*(y-2.1)) ~ QS0 + QS1*y + QS2*y^2
QS = (0.79198033, 0.09684914, -0.18654798)
# lp(y) = log1p(2^(y-2.1)) ~ QR0 + QR1*y + QR2*y^2
QR = (0.20992082, 0.12795303, 0.04476109)

F32 = mybir.dt.float32
BF16 = mybir.dt.bfloat16
AF = mybir.ActivationFunctionType
ALU = mybir.AluOpType

_CACHE = {}


def _make_dve_op(name, spec):
    """Register a custom DVE op at runtime (name -> free opcode row)."""
    existing = {op.name: op for op in dops.OPS}
    if name in existing:
        return existing[name]
    row = dops._CUSTOM_DVE_ROW_BASE + len(dops.OPS)
    tmp = DveOpSpec(name=name, opcode=row, uops=lower(spec, ver="v3"),
                    rd1_en=_has_src1(spec))
    op = dops.DveOp(name, spec, subdim=False, uops_sha={"v3": tmp.sha("v3")})
    dops.OPS.append(op)
    dops._SUB_OPCODE_FOR_NAME[name] = row
    dops.CUSTOM_DVE_SPECS[name] = spec
    return op


def _get_ops():
    if "ops" in _CACHE:
        return _CACHE["ops"]
    # ds = max(|x - y|, eps) - 0.5
    absdiff = _make_dve_op(
        "AWL_ABSDIFF_SHIFT",
        Spec(
            body=maxx(maxx(Src0 - Src1, Src1 - Src0), C1) - C0,
            reference=lambda in0, in1, s0, s1, imm2: (
                np.maximum(np.abs(in0.astype(np.float32) - in1), s1) - s0
            ),
        ),
    )
    # accum += relu(ds) * (q0 + q1*yy + q2*yy^2); ds=Src0, yy=Src1
    lin_red = _make_dve_op(
        "AWL_LIN_REDUCE2",
        Spec(
            body=relu(Src0) * ((C2 * Src1 + C1) * Src1 + C0),
            accum=dops.add,
            accum_init=Zero,
            reference=lambda in0, in1, s0, s1, imm2: (
                lambda b: (b, b.reshape(b.shape[0], -1).sum(-1, keepdims=True))
            )(
                np.maximum(in0.astype(np.float32), 0)
                * ((imm2 * in1 + s1) * in1 + s0)
            ),
        ),
    )
    # accum += min((r0 + r1*pm + r2*pm^2) - nl, 0); nl=Src0, pm=Src1
    lp_min_red = _make_dve_op(
        "AWL_LPMIN_REDUCE",
        Spec(
            body=minn(((C2 * Src1 + C1) * Src1 + C0) - Src0, Zero),
            accum=dops.add,
            accum_init=Zero,
            reference=lambda in0, in1, s0, s1, imm2: (
                lambda b: (b, b.reshape(b.shape[0], -1).sum(-1, keepdims=True))
            )(
                np.minimum((imm2 * in1 + s1) * in1 + s0 - in0.astype(np.float32), 0)
            ),
        ),
    )
    _CACHE["ops"] = (absdiff, lin_red, lp_min_red)
    return _CACHE["ops"]


def _register_const(nc, value, dtype=F32):
    t = nc.alloc_sbuf_tensor(f"const-{dtype.name}-{value}", [128, 1], dtype)
    nc.gpsimd.memset(t.ap(), value)
    nc.const_aps.aps[(dtype, value)] = t.ap()


def _pin_act_table():
    """Force every ACTIVATE onto the combined {ln, exp} table so the
    compiler never inserts per-instruction ACT_TABLE_LOAD switches."""
    if _CACHE.get("act_pinned"):
        return
    orig = bacc.get_activation_tables
    keep = "natural_log_exp_and_others"

    def patched(module_arch):
        tables = dict(orig(module_arch))
        return {k: (v if k == keep else set()) for k, v in tables.items()}

    bacc.get_activation_tables = patched
    _CACHE["act_pinned"] = True


def _build():
    absdiff, lin_red, lp_min_red = _get_ops()
    _pin_act_table()
    nc = bacc.Bacc(None, target_bir_lowering=False)
    _register_const(nc, 0.5)
    nc.all_engine_barrier()
    x_ext = nc.declare_dram_parameter("x", [ROWS, COLS], BF16, isOutput=False)
    y_ext = nc.declare_dram_parameter("y", [ROWS, COLS], BF16, isOutput=False)
    out_ext = nc.declare_dram_parameter("out", [128, 1], F32, isOutput=True)

    with TileContext(nc) as tc:
        with (
            tc.tile_pool(name="io", bufs=4) as iop,
            tc.tile_pool(name="work", bufs=4) as wp,
            tc.tile_pool(name="accp", bufs=1) as accp,
        ):
            NCH = NT + 2
            accN = accp.tile([128, NCH], F32, tag="accN")
            accA = accp.tile([128, NCH], F32, tag="accA")
            accB = accp.tile([128, NCH], F32, tag="accB")

            chunks = []
            for t in range(NT):
                if t in (0, NT - 1):
                    chunks += [(t, 0, COLS // 2), (t, COLS // 2, COLS // 2)]
                else:
                    chunks.append((t, 0, COLS))
            for ci, (t, c0, fd) in enumerate(chunks):
                xt = iop.tile([128, COLS], BF16, tag="x", name=f"x_{ci}")
                yt = iop.tile([128, COLS], BF16, tag="y", name=f"y_{ci}")
                xt, yt = xt[:, :fd], yt[:, :fd]
                nc.sync.dma_start(out=xt, in_=x_ext[t * 128:(t + 1) * 128, c0:c0 + fd])
                nc.sync.dma_start(out=yt, in_=y_ext[t * 128:(t + 1) * 128, c0:c0 + fd])

                ds = wp.tile([128, COLS], BF16, tag="ds", name=f"ds_{ci}")[:, :fd]
                nc.vector._custom_dve(absdiff, out=ds, in0=xt, in1=yt,
                                      s0=0.5, s1=D_EPS)
                junkA = wp.tile([128, COLS], BF16, tag="junk", name=f"junkA_{ci}")[:, :fd]
                nc.vector._custom_dve(lin_red, out=junkA, in0=ds, in1=yt,
                                      s0=QS[0], s1=QS[1], imm2=QS[2],
                                      accum_out=accA[:, ci:ci + 1])
                ld = wp.tile([128, COLS], BF16, tag="ld", name=f"ld_{ci}")[:, :fd]
                nc.scalar.activation(ld, ds, AF.Ln, bias=0.5)
                pm = wp.tile([128, COLS], BF16, tag="pm", name=f"pm_{ci}")[:, :fd]
                nc.vector.tensor_scalar(pm, yt, -2.1, None, ALU.add)
                w = wp.tile([128, COLS], BF16, tag="w", name=f"w_{ci}")[:, :fd]
                nc.vector.tensor_tensor(w, pm, ld, ALU.mult)
                e2 = wp.tile([128, COLS], BF16, tag="e2", name=f"e2_{ci}")[:, :fd]
                nc.scalar.activation(e2, w, AF.Exp, scale=-1.0)
                nl = wp.tile([128, COLS], BF16, tag="nl", name=f"nl_{ci}")[:, :fd]
                nc.scalar.activation(nl, e2, AF.Ln, bias=1.0,
                                     accum_out=accN[:, ci:ci + 1])
                junkB = wp.tile([128, COLS], BF16, tag="junk", name=f"junkB_{ci}")[:, :fd]
                nc.vector._custom_dve(lp_min_red, out=junkB, in0=nl, in1=yt,
                                      s0=QR[0], s1=QR[1], imm2=QR[2],
                                      accum_out=accB[:, ci:ci + 1])

            rN = accp.tile([128, 1], F32, tag="rN")
            nc.vector.tensor_reduce(rN[:, :], accN[:, :], mybir.AxisListType.X, ALU.add)
            rA = accp.tile([128, 1], F32, tag="rA")
            nc.vector.tensor_reduce(rA[:, :], accA[:, :], mybir.AxisListType.X, ALU.add)
            rB = accp.tile([128, 1], F32, tag="rB")
            nc.vector.tensor_reduce(rB[:, :], accB[:, :], mybir.AxisListType.X, ALU.add)
            s1 = accp.tile([128, 1], F32, tag="s1")
            nc.vector.tensor_tensor(s1[:, :], rN[:, :], rA[:, :], ALU.add)
            s2 = accp.tile([128, 1], F32, tag="s2")
            nc.vector.tensor_tensor(s2[:, :], s1[:, :], rB[:, :], ALU.add)
            nc.sync.dma_start(out=out_ext[:, :], in_=s2[:, :])

    nc.compile()
    return nc


def _get_nc():
    if "nc" not in _CACHE:
        _CACHE["nc"] = _build()
    return _CACHE["nc"]


def kernel(input, target):
    import ml_dtypes
    x = np.ascontiguousarray(input, dtype=np.float32).reshape(N_CORES, ROWS, COLS).astype(ml_dtypes.bfloat16)
    y = np.ascontiguousarray(target, dtype=np.float32).reshape(N_CORES, ROWS, COLS).astype(ml_dtypes.bfloat16)
    nc = _get_nc()
    in_maps = [{"x": x[i], "y": y[i]} for i in range(N_CORES)]
    res = run_bass_kernel_spmd(nc, in_maps, core_ids=list(range(N_CORES)))
    total = sum(float(res.results[i]["out"].sum()) for i in range(N_CORES))
    return np.float32(14.0 * total)


# revision 18
# speedup vs baseline: 1.2285x; 1.2285x over previous
"""AdaptiveWingLoss on 8 TRN2 NeuronCores.

Math (theta=0.5, eps=1, alpha=2.1, omega=14):
  d  = |x - y|,  p = 2.1 - y,  pm = y - 2.1,  z = ln2*pm
  nl  = log1p(d^p) = Ln(exp(p*ln d) + 1)
  lp  = log1p(e^z),  sigma = 1/(1+e^-z),  ps = p*sigma
  lin = ps*(2d-1) + lp = A'(y)*(d-0.5) + lp,  A' = 2*ps = -2*pm*sigma(z)
  loss/14 = select(d<0.5, nl, lin)

Key identity: d >= 0.5  <=>  nl >= lp (monotone), so the select dissolves:
  sum(loss)/14 = sum(nl) + sum(A' * relu(d-0.5)) + sum(min(lp - nl, 0))

sigma(z) and lp are approximated by quadratics in pm (max rel err ~1e-3,
end-to-end ~3e-6), evaluated inside fused custom DVE ops with accum=ADD.
All ACT work is in the single {ln, exp} table set (no table switches).
Batch dim sharded across 8 cores; each core emits per-partition partial
sums [128,1]; host adds and scales by 14.
"""
import numpy as np

import concourse.bacc as bacc
import concourse.mybir as mybir
import concourse.dve_ops as dops
from concourse.dve_spec import Spec, Src0, Src1, C0, C1, C2, Zero, lower, maxx, minn, relu, _has_src1
from concourse.dve_uop import DveOpSpec
from concourse.tile import TileContext
from concourse.bass_utils import run_bass_kernel_spmd

N_CORES = 8
ROWS, COLS = 1024, 2048  # per-core shard, fp32 elements
NT = ROWS // 128
LN2 = float(np.log(2.0))
D_EPS = 1e-6

# quadratic fits in y on [0, 1] (np.polyfit deg 2; end-to-end err ~5e-5)
# A'(y) = -2*(y-2.1)*sigma(ln2*(y-2.1)) ~ QS0 + QS1*y + QS2*y^2
QS = (0.79198033, 0.09684914, -0.18654798)
# lp(y) = log1p(2^(y-2.1)) ~ QR0 + QR1*y + QR2*y^2
QR = (0.20992082, 0.12795303, 0.04476109)

F32 = mybir.dt.float32
BF16 = mybir.dt.bfloat16
AF = mybir.ActivationFunctionType
ALU = mybir.AluOpType

_CACHE = {}


def _make_dve_op(name, spec):
    """Register a custom DVE op at runtime (name -> free opcode row)."""
    existing = {op.name: op for op in dops.OPS}
    if name in existing:
        return existing[name]
    row = dops._CUSTOM_DVE_ROW_BASE + len(dops.OPS)
    tmp = DveOpSpec(name=name, opcode=row, uops=lower(spec, ver="v3"),
                    rd1_en=_has_src1(spec))
    op = dops.DveOp(name, spec, subdim=False, uops_sha={"v3": tmp.sha("v3")})
    dops.OPS.append(op)
    dops._SUB_OPCODE_FOR_NAME[name] = row
    dops.CUSTOM_DVE_SPECS[name] = spec
    return op


def _get_ops():
    if "ops" in _CACHE:
        return _CACHE["ops"]
    # ds = max(|x - y|, eps) - 0.5
    absdiff = _make_dve_op(
        "AWL_ABSDIFF_SHIFT",
        Spec(
            body=maxx(maxx(Src0 - Src1, Src1 - Src0), C1) - C0,
            reference=lambda in0, in1, s0, s1, imm2: (
                np.maximum(np.abs(in0.astype(np.float32) - in1), s1) - s0
            ),
        ),
    )
    # accum += relu(ds) * (q0 + q1*yy + q2*yy^2); ds=Src0, yy=Src1
    lin_red = _make_dve_op(
        "AWL_LIN_REDUCE2",
        Spec(
            body=relu(Src0) * ((C2 * Src1 + C1) * Src1 + C0),
            accum=dops.add,
            accum_init=Zero,
            reference=lambda in0, in1, s0, s1, imm2: (
                lambda b: (b, b.reshape(b.shape[0], -1).sum(-1, keepdims=True))
            )(
                np.maximum(in0.astype(np.float32), 0)
                * ((imm2 * in1 + s1) * in1 + s0)
            ),
        ),
    )
    # accum += min((r0 + r1*pm + r2*pm^2) - nl, 0); nl=Src0, pm=Src1
    lp_min_red = _make_dve_op(
        "AWL_LPMIN_REDUCE",
        Spec(
            body=minn(((C2 * Src1 + C1) * Src1 + C0) - Src0, Zero),
            accum=dops.add,
            accum_init=Zero,
            reference=lambda in0, in1, s0, s1, imm2: (
                lambda b: (b, b.reshape(b.shape[0], -1).sum(-1, keepdims=True))
            )(
                np.minimum((imm2 * in1 + s1) * in1 + s0 - in0.astype(np.float32), 0)
            ),
        ),
    )
    _CACHE["ops"] = (absdiff, lin_red, lp_min_red)
    return _CACHE["ops"]


def _register_const(nc, value, dtype=F32):
    t = nc.alloc_sbuf_tensor(f"const-{dtype.name}-{value}", [128, 1], dtype)
    nc.gpsimd.memset(t.ap(), value)
    nc.const_aps.aps[(dtype, value)] = t.ap()


def _pin_act_table():
    """Force every ACTIVATE onto the combined {ln, exp} table so the
    compiler never inserts per-instruction ACT_TABLE_LOAD switches."""
    if _CACHE.get("act_pinned"):
        return
    orig = bacc.get_activation_tables
    keep = "natural_log_exp_and_others"

    def patched(module_arch):
        tables = dict(orig(module_arch))
        return {k: (v if k == keep else set()) for k, v in tables.items()}

    bacc.get_activation_tables = patched
    _CACHE["act_pinned"] = True


def _patch_tile_tail():
    if _CACHE.get("tail_patched"):
        return
    from concourse.tile import TileContext as _TC

    def _drain_and_barrier(self, tick_clock, wait_clock):
        from concourse.tile import ScopedClock
        drain_inst = self.nc.sync.drain()
        wait_clock.add_sem_waits(
            drain_inst.ins, ScopedClock({None: tick_clock.global_clock})
        )
        self.nc.all_engine_barrier()
        popped = self.nc._tile_sem_poison_stack.pop()
        assert popped is self._sem_poison

    _TC._drain_and_barrier = _drain_and_barrier
    _CACHE["tail_patched"] = True


def _build():
    absdiff, lin_red, lp_min_red = _get_ops()
    _pin_act_table()
    _patch_tile_tail()
    nc = bacc.Bacc(None, target_bir_lowering=False)
    _register_const(nc, 0.5)
    nc.all_engine_barrier()
    x_ext = nc.declare_dram_parameter("x", [ROWS, COLS], BF16, isOutput=False)
    y_ext = nc.declare_dram_parameter("y", [ROWS, COLS], BF16, isOutput=False)
    out_ext = nc.declare_dram_parameter("out", [128, 1], F32, isOutput=True)

    with TileContext(nc) as tc:
        with (
            tc.tile_pool(name="io", bufs=4) as iop,
            tc.tile_pool(name="work", bufs=4) as wp,
            tc.tile_pool(name="accp", bufs=1) as accp,
        ):
            NCH = NT
            accN = accp.tile([128, NCH], F32, tag="accN")
            accA = accp.tile([128, NCH], F32, tag="accA")
            accB = accp.tile([128, NCH], F32, tag="accB")

            chunks = [(t, 0, COLS) for t in range(NT)]
            for ci, (t, c0, fd) in enumerate(chunks):
                xt = iop.tile([128, COLS], BF16, tag="x", name=f"x_{ci}")
                yt = iop.tile([128, COLS], BF16, tag="y", name=f"y_{ci}")
                xt, yt = xt[:, :fd], yt[:, :fd]
                nc.sync.dma_start(out=xt, in_=x_ext[t * 128:(t + 1) * 128, c0:c0 + fd])
                nc.sync.dma_start(out=yt, in_=y_ext[t * 128:(t + 1) * 128, c0:c0 + fd])

                ds = wp.tile([128, COLS], BF16, tag="ds", name=f"ds_{ci}")[:, :fd]
                nc.vector._custom_dve(absdiff, out=ds, in0=xt, in1=yt,
                                      s0=0.5, s1=D_EPS)
                junkA = wp.tile([128, COLS], BF16, tag="junk", name=f"junkA_{ci}")[:, :fd]
                nc.vector._custom_dve(lin_red, out=junkA, in0=ds, in1=yt,
                                      s0=QS[0], s1=QS[1], imm2=QS[2],
                                      accum_out=accA[:, ci:ci + 1])
                ld = wp.tile([128, COLS], BF16, tag="ld", name=f"ld_{ci}")[:, :fd]
                nc.scalar.activation(ld, ds, AF.Ln, bias=0.5)
                pm = wp.tile([128, COLS], BF16, tag="pm", name=f"pm_{ci}")[:, :fd]
                nc.vector.tensor_scalar(pm, yt, -2.1, None, ALU.add)
                w = wp.tile([128, COLS], BF16, tag="w", name=f"w_{ci}")[:, :fd]
                nc.vector.tensor_tensor(w, pm, ld, ALU.mult)
                e2 = wp.tile([128, COLS], BF16, tag="e2", name=f"e2_{ci}")[:, :fd]
                nc.scalar.activation(e2, w, AF.Exp, scale=-1.0)
                nl = wp.tile([128, COLS], BF16, tag="nl", name=f"nl_{ci}")[:, :fd]
                nc.scalar.activation(nl, e2, AF.Ln, bias=1.0,
                                     accum_out=accN[:, ci:ci + 1])
                junkB = wp.tile([128, COLS], BF16, tag="junk", name=f"junkB_{ci}")[:, :fd]
                nc.vector._custom_dve(lp_min_red, out=junkB, in0=nl, in1=yt,
                                      s0=QR[0], s1=QR[1], imm2=QR[2],
                                      accum_out=accB[:, ci:ci + 1])

            rN = accp.tile([128, 1], F32, tag="rN")
            nc.vector.tensor_reduce(rN[:, :], accN[:, :], mybir.AxisListType.X, ALU.add)
            rA = accp.tile([128, 1], F32, tag="rA")
            nc.vector.tensor_reduce(rA[:, :], accA[:, :], mybir.AxisListType.X, ALU.add)
            rB = accp.tile([128, 1], F32, tag="rB")
            nc.vector.tensor_reduce(rB[:, :], accB[:, :], mybir.AxisListType.X, ALU.add)
            s1 = accp.tile([128, 1], F32, tag="s1")
            nc.vector.tensor_tensor(s1[:, :], rN[:, :], rA[:, :], ALU.add)
            s2 = accp.tile([128, 1], F32, tag="s2")
            nc.vector.tensor_tensor(s2[:, :], s1[:, :], rB[:, :], ALU.add)
            nc.sync.dma_start(out=out_ext[:, :], in_=s2[:, :])

    nc.compile()
    return nc


def _get_nc():
    if "nc" not in _CACHE:
        _CACHE["nc"] = _build()
    return _CACHE["nc"]


def kernel(input, target):
    import ml_dtypes
    x = np.ascontiguousarray(input, dtype=np.float32).reshape(N_CORES, ROWS, COLS).astype(ml_dtypes.bfloat16)
    y = np.ascontiguousarray(target, dtype=np.float32).reshape(N_CORES, ROWS, COLS).astype(ml_dtypes.bfloat16)
    nc = _get_nc()
    in_maps = [{"x": x[i], "y": y[i]} for i in range(N_CORES)]
    res = run_bass_kernel_spmd(nc, in_maps, core_ids=list(range(N_CORES)))
    total = sum(float(res.results[i]["out"].sum()) for i in range(N_CORES))
    return np.float32(14.0 * total)
